# revision 82
# baseline (speedup 1.0000x reference)
"""Trainium2 Bass kernel for windowed/global sparse attention (Swin-style
relative-position bias + 1 global token), data-parallel over batch on 8 cores.

Shapes: B=16, N=785 (1 global + 28x28 local), C=768, H=12 heads, d=64.

Per-core device program (2 batches/core, software-pipelined):
  - qT/kT computed transposed ([d, tokens]) so S^T = K @ Q^T needs no
    transposes anywhere; v computed natural ([tokens, d]) with a ones column
    appended per head so the P @ V matmul also yields softmax denominators.
  - softmax: exp(S + bias) = exp(S) * expB with expB = exp(bias) gathered on
    host at constant indices and shipped bf16; the two heads of a pair write
    one fused [128, 2W] SBUF exp tile so the expB multiply is a single
    2x-rate DVE op.
  - PSUM: 3 rotating 2-bank slots for S tiles (and qkv convoys) + one
    dedicated slot for O/v/proj convoys, decoupling the S stream from the O
    convoys.
  - normalization: denominators from all 12 heads staged to DRAM, one batched
    DVE reciprocal, DMA-broadcast back to [128, N], multiplied into O^T;
    proj consumes O^T directly as lhsT.
  - schedule: [qkv0 || v0] dense, then attention-0 with x1/qkv1/v1 as PE
    gap-filler, attention-1 with proj0 as filler, then norm1+proj1 - keeps
    the PE activity monitor from re-throttling the clock during the
    exp-paced attention stretches.
"""

import numpy as np
import ml_dtypes

import concourse.bass as bass
import concourse.bacc as bacc
import concourse.tile as tile
from concourse.tile import add_dep_helper
from concourse import mybir
from concourse.bass_utils import run_bass_kernel_spmd

F32 = mybir.dt.float32
BF16 = mybir.dt.bfloat16

WX = WY = 28
NGLO = 1
H = 12
L = WX * WY            # 784
N = NGLO + L           # 785
C = 768
HD = C // H            # 64
SCALE = HD ** -0.5
B = 16
N_CORES = 8
B_LOC = B // N_CORES   # 2
NCC = C // 128         # 6 contraction chunks
NKC = (N + 127) // 128  # 7 key/token chunks (last = 17 rows)
NPAIR = H // 2         # 6 head pairs
W = 786                # padded free width for N-sized tiles (even, 4B-aligned)
W2 = 2 * W

CG_N = [(0, 512), (512, 274)]
CG_C = [(0, 512), (512, 256)]


def _kr(kc):
    return min(128, N - kc * 128)


def build_nc():
    nc = bacc.Bacc(None, target_bir_lowering=False)

    xT_d = nc.dram_tensor("xT", [B_LOC, C, N], BF16, kind="ExternalInput")
    qkvwT_d = nc.dram_tensor("qkv_wT", [C, 3 * C], BF16, kind="ExternalInput")
    pwT_d = nc.dram_tensor("proj_wT", [C, C], BF16, kind="ExternalInput")
    pb_d = nc.dram_tensor("proj_b", [1, C], BF16, kind="ExternalInput")
    expB_d = nc.dram_tensor("expB", [H, N, N], BF16, kind="ExternalInput")
    out_d = nc.dram_tensor("out", [B_LOC, N, C], BF16, kind="ExternalOutput")
    dinv_d = nc.dram_tensor("dinv_scratch", [B_LOC, H, N], BF16)

    with tile.TileContext(nc) as tc:
        with (
            tc.tile_pool(name="consts", bufs=1) as consts,
            tc.tile_pool(name="perb", bufs=2) as perb,
            tc.tile_pool(name="expbp", bufs=3) as expbp,
            tc.tile_pool(name="flow", bufs=4) as flow,
            tc.tile_pool(name="ptp", bufs=8) as ptp,
            tc.tile_pool(name="norm", bufs=1) as norm,
            tc.tile_pool(name="outp", bufs=2) as outp,
            tc.tile_pool(name="psum_s", bufs=3, space=bass.MemorySpace.PSUM) as psum_s,
            tc.tile_pool(name="psum_o", bufs=1, space=bass.MemorySpace.PSUM) as psum_o,
        ):
            # ---- weights (resident, bf16); proj weights loaded last ----
            qkvw = []
            for cc in range(NCC):
                t = consts.tile([128, 3 * C], BF16, tag=f"qkvw{cc}", name=f"qkvw{cc}")
                qkvw.append(t)
            pw16 = []
            for cc in range(NCC):
                t = consts.tile([128, C], BF16, tag=f"pw{cc}", name=f"pw{cc}")
                pw16.append(t)
            pb_rep = consts.tile([128, C], BF16, tag="pbrep")

            def emit_weight_loads_proj():
                # proj weights aren't needed until phase C; issue them on the
                # Pool queue to keep sync free for the expB stream.
                for cc in range(NCC):
                    nc.gpsimd.dma_start(
                        pw16[cc][:], pwT_d[cc * 128:(cc + 1) * 128, :]
                    )
                nc.gpsimd.dma_start(pb_rep[:], pb_d[:].to_broadcast([128, C]))

            def emit_x(b, eng=None):
                # pad column [N:W] left as garbage: it only ever feeds the
                # q=785 / token=785 output columns, which are never read.
                eng = eng or nc.sync
                xts = []
                for cc in range(NCC):
                    t = perb.tile([128, W], BF16, tag=f"xt{cc}", name=f"xt{cc}_{b}")
                    eng.dma_start(
                        t[:, 0:N], xT_d[b, cc * 128:(cc + 1) * 128, :]
                    )
                    xts.append(t)
                return xts

            def emit_x0_and_qkvw_interleaved():
                # startup critical path: the first qkv matmuls need the q/k
                # weight columns and x chunks in cc order; v columns aren't
                # touched until the first v convoy, so defer them. Interleave
                # so chunk 0 of everything lands first.
                xts = []
                for cc in range(NCC):
                    nc.sync.dma_start(
                        qkvw[cc][:], qkvwT_d[cc * 128:(cc + 1) * 128, :]
                    )
                    t = perb.tile([128, W], BF16, tag=f"xt{cc}", name=f"xt{cc}_0")
                    nc.scalar.dma_start(
                        t[:, 0:N], xT_d[0, cc * 128:(cc + 1) * 128, :]
                    )
                    xts.append(t)
                return xts

            def emit_qkvT_chunk(b, xts, j, qT, kT, evac_vector=False,
                                defer=False, ocs=None):
                """produce qT[j] and kT[j] for batch b."""
                firsts = []
                evacs = []
                for oc in (ocs if ocs is not None else (j, NCC + j)):
                    ps = psum_s.tile([128, W], F32, tag="s", name=f"psqk{oc}_{b}")
                    for cc in range(NCC):
                        for (c0, cn) in CG_N:
                            mm = nc.tensor.matmul(
                                ps[:, c0:c0 + cn],
                                qkvw[cc][:, oc * 128:(oc + 1) * 128],
                                xts[cc][:, c0:c0 + cn],
                                start=(cc == 0),
                                stop=(cc == NCC - 1),
                            )
                            if cc == 0 and c0 == 0:
                                firsts.append(mm)
                    dst = qT[oc] if oc < NCC else kT[oc - NCC]

                    def ev(dst=dst, ps=ps):
                        if evac_vector:
                            nc.vector.tensor_copy(dst[:, 0:W], ps[:, 0:W])
                        else:
                            nc.scalar.copy(dst[:, 0:W], ps[:, 0:W])
                    if defer:
                        evacs.append(ev)
                    else:
                        ev()
                if defer:
                    return firsts, evacs
                return firsts

            def emit_v_convoy(b, xts, kc, vp, evac_vector, defer=False):
                """one key-chunk's V matmul convoy + evac into vp[kc]."""
                kr = _kr(kc)
                ps = psum_o.tile([128, C], F32, tag="o", name=f"psv{kc}_{b}")
                first = None
                for cc in range(NCC):
                    for (c0, cn) in CG_C:
                        mm = nc.tensor.matmul(
                            ps[0:kr, c0:c0 + cn],
                            xts[cc][:, kc * 128:kc * 128 + kr],
                            qkvw[cc][:, 2 * C + c0:2 * C + c0 + cn],
                            start=(cc == 0),
                            stop=(cc == NCC - 1),
                        )
                        if first is None:
                            first = mm
                v3 = vp[kc][:].rearrange("p (h e) -> p h e", e=HD + 1)

                def ev():
                    if evac_vector:
                        nc.vector.tensor_copy(
                            v3[0:kr, :, 0:HD],
                            ps[0:kr, :].rearrange("p (h d) -> p h d", d=HD),
                        )
                    else:
                        nc.scalar.copy(
                            v3[0:kr, :, 0:HD],
                            ps[0:kr, :].rearrange("p (h d) -> p h d", d=HD),
                        )
                    nc.gpsimd.memset(v3[0:kr, :, HD:HD + 1], 1.0)
                if defer:
                    return first, ev
                ev()
                return first

            def alloc_vp(b):
                return [perb.tile([128, H * (HD + 1)], BF16, tag=f"vp{i}",
                                  name=f"vp{i}_{b}") for i in range(NKC)]

            def alloc_oT(b):
                return [perb.tile([128, W], BF16, tag=f"oT{i}", name=f"oT{i}_{b}")
                        for i in range(NCC)]

            def emit_attn_S(b, j, qT, kT, kcs, store, pacer_box):
                """S matmuls + ebt DMA only (PE + sync streams) for head pair
                (2j, 2j+1); exp/mult emitted separately so PE-ready S work
                can be emitted ahead of O convoys without perturbing the
                ACT/DVE instruction order."""
                for kc in kcs:
                    kr = _kr(kc)
                    ps_pair = [
                        psum_s.tile([128, W], F32, tag="s",
                                    name=f"pss{2 * j + hh}_{kc}_{b}")
                        for hh in range(2)
                    ]
                    for (c0, cn) in CG_N:
                        for hh in range(2):
                            po = hh * 64
                            mm = nc.tensor.matmul(
                                ps_pair[hh][0:kr, c0:c0 + cn],
                                kT[j][po:po + 64, kc * 128:kc * 128 + kr],
                                qT[j][po:po + 64, c0:c0 + cn],
                                start=True,
                                stop=True,
                            )
                            if kc == 2 and pacer_box[0] is None:
                                pacer_box[0] = mm
                    ebt = expbp.tile([128, W2], BF16, tag="expb",
                                     name=f"ebt{j}_{kc}_{b}")
                    # one fused DMA for both heads of the pair: dst viewed as
                    # [kr, 2, W], src as [kr, 2, N] — halves HWDGE issue work
                    ebt3 = ebt[0:kr, :].rearrange("k (h w) -> k h w", w=W)
                    src3 = expB_d[
                        2 * j:2 * j + 2, kc * 128:kc * 128 + kr, :
                    ].rearrange("h k n -> k h n")
                    nc.sync.dma_start(ebt3[:, :, 0:N], src3)
                    store[kc] = (ps_pair, ebt)

            def emit_attn_expmult(b, j, kcs, store, pts):
                """exp (ACT) + fused expB multiply (DVE) for staged S tiles."""
                for kc in kcs:
                    kr = _kr(kc)
                    ps_pair, ebt = store[kc]
                    es = flow.tile([128, W2], BF16, tag="expS",
                                   name=f"es{j}_{kc}_{b}")
                    for hh in range(2):
                        nc.scalar.activation(
                            es[0:kr, hh * W:(hh + 1) * W],
                            ps_pair[hh][0:kr, 0:W],
                            mybir.ActivationFunctionType.Exp,
                        )
                    pt = ptp.tile([128, W2], BF16, tag="pT",
                                  name=f"pt{j}_{kc}_{b}")
                    nc.vector.tensor_tensor(
                        pt[0:kr, 0:W2],
                        es[0:kr, 0:W2],
                        ebt[0:kr, 0:W2],
                        mybir.AluOpType.mult,
                    )
                    pts[kc] = pt

            def emit_attn_pass1(b, j, qT, kT, kcs, pts, pacer_box):
                store = {}
                emit_attn_S(b, j, qT, kT, kcs, store, pacer_box)
                emit_attn_expmult(b, j, kcs, store, pts)

            def emit_tail_group(b, g, qT, kT):
                """kc=6 tail (17 k-rows) for three heads 3g..3g+2, packed at
                partition stripes {0,32,64} of ONE psum tile: one exp and
                one expB-multiply instead of three of each. Gap stripes hold
                garbage that is never read downstream."""
                ps_t = psum_s.tile([128, W], F32, tag="s",
                                   name=f"pstail{g}_{b}")
                for idx in range(3):
                    h = 3 * g + idx
                    j, po, p0 = h // 2, 64 * (h % 2), 32 * idx
                    for (c0, cn) in CG_N:
                        nc.tensor.matmul(
                            ps_t[p0:p0 + 17, c0:c0 + cn],
                            kT[j][po:po + 64, 6 * 128:N],
                            qT[j][po:po + 64, c0:c0 + cn],
                            start=True,
                            stop=True,
                        )
                ebt_t = expbp.tile([128, W], BF16, tag="expbt", bufs=1,
                                   name=f"ebtail{g}_{b}")
                for idx in range(3):
                    h = 3 * g + idx
                    p0 = 32 * idx
                    nc.sync.dma_start(
                        ebt_t[p0:p0 + 17, 0:N], expB_d[h, 6 * 128:N, :]
                    )
                es_t = flow.tile([128, W], BF16, tag="expSt", bufs=1,
                                 name=f"estail{g}_{b}")
                nc.scalar.activation(
                    es_t[0:81, 0:W], ps_t[0:81, 0:W],
                    mybir.ActivationFunctionType.Exp,
                )
                pt_t = ptp.tile([128, W], BF16, tag="pTt", bufs=2,
                                name=f"pttail{g}_{b}")
                nc.vector.tensor_tensor(
                    pt_t[0:81, 0:W], es_t[0:81, 0:W], ebt_t[0:81, 0:W],
                    mybir.AluOpType.mult,
                )
                return pt_t

            def emit_vp6_stripes(b, vp):
                # replicate the 17 tail V rows (and their ones column) to the
                # 32-aligned stripes the packed tail P tiles live at, so the
                # kc=6 O matmuls see matching operand partition bases
                for idx in range(1, 3):
                    nc.sync.dma_start(
                        vp[6][32 * idx:32 * idx + 17, :], vp[6][0:17, :]
                    )

            def emit_attn_pass2(b, j, pts, ptails, vp, oT, dall, heads=(0, 1)):
                """dense O-accumulation convoy for head pair (2j, 2j+1).
                kc<6 P tiles are per-pair [128, 2W]; the kc=6 tail P comes
                from the packed 3-head tile ptails[h//3] at stripe 32*(h%3).
                `heads` selects which of the pair's heads to emit, so the
                two convoys can be interleaved with other PE work."""
                for hh in heads:
                    h = 2 * j + hh
                    p0 = 32 * (h % 3)
                    ptail = ptails[h // 3]
                    ps_o = psum_o.tile([128, W], F32, tag="o",
                                       name=f"pso{h}_{b}")
                    for kc in range(NKC):
                        kr = _kr(kc)
                        for (c0, cn) in CG_N:
                            if kc < NKC - 1:
                                lhsT = vp[kc][0:kr,
                                              h * (HD + 1):(h + 1) * (HD + 1)]
                                rhs = pts[kc][0:kr,
                                              hh * W + c0:hh * W + c0 + cn]
                            else:
                                lhsT = vp[kc][p0:p0 + kr,
                                              h * (HD + 1):(h + 1) * (HD + 1)]
                                rhs = ptail[p0:p0 + kr, c0:c0 + cn]
                            nc.tensor.matmul(
                                ps_o[0:HD + 1, c0:c0 + cn],
                                lhsT,
                                rhs,
                                start=(kc == 0),
                                stop=(kc == NKC - 1),
                            )
                    if hh == 0:
                        # denominator row rides along in the oT evac (row 64
                        # is head B's territory, but the dall DMA reads it
                        # before head B's evac overwrites — WAR-ordered)
                        nc.vector.tensor_copy(
                            oT[j][0:65, 0:N], ps_o[0:65, 0:N]
                        )
                        nc.sync.dma_start(
                            dall[h:h + 1, 0:N], oT[j][64:65, 0:N]
                        )
                    else:
                        nc.vector.tensor_copy(
                            oT[j][64:128, 0:N], ps_o[0:64, 0:N]
                        )
                        dn = norm.tile([65, W], BF16, tag="dn", bufs=1,
                                       name=f"dn{h}_{b}")
                        nc.vector.tensor_copy(dn[64:65, 0:N],
                                              ps_o[64:65, 0:N])
                        nc.sync.dma_start(dall[h:h + 1, 0:N],
                                          dn[64:65, 0:N])

            def emit_norm_recip(b, dall, pj0, pj1, scalar_recip=False):
                """1/den for pairs [pj0, pj1) into a bf16 dinv tile.
                DVE path: cast->recip_approx->cast (no ACT involvement).
                ACT path (tail only, ACT drained): 1/x = exp(-ln x)."""
                # engines need 32-aligned partition bases; recomputing rows
                # 0..h0 is free (cost is free-size-bound), so start at 0
                h0, h1 = 0, 2 * pj1
                dinv16 = norm.tile([12, W], BF16, tag="dinv16",
                                   name=f"dinv16_{pj0}_{b}", bufs=1)
                if scalar_recip:
                    lnt = norm.tile([12, W], F32, tag="dall32", bufs=1,
                                    name=f"lnt_{pj0}_{b}")
                    nc.scalar.activation(
                        lnt[h0:h1, 0:N], dall[h0:h1, 0:N],
                        mybir.ActivationFunctionType.Ln,
                    )
                    nc.scalar.activation(
                        dinv16[h0:h1, 0:N], lnt[h0:h1, 0:N],
                        mybir.ActivationFunctionType.Exp,
                        scale=-1.0,
                    )
                else:
                    dall32 = norm.tile([12, W], F32, tag="dall32",
                                       name=f"dall32_{pj0}_{b}", bufs=1)
                    dinv32 = norm.tile([12, W], F32, tag="dinv32",
                                       name=f"dinv32_{pj0}_{b}", bufs=1)
                    nc.vector.tensor_copy(dall32[h0:h1, 0:N],
                                          dall[h0:h1, 0:N])
                    nc.vector.reciprocal_approx_fast(
                        dinv32[h0:h1, 0:N], dall32[h0:h1, 0:N])
                    with nc.allow_low_precision(
                            reason="1/den in bf16: uniform per-row scale, "
                                   "~0.2% rms, well inside the 2e-2 gate"):
                        nc.vector.tensor_copy(dinv16[h0:h1, 0:N],
                                              dinv32[h0:h1, 0:N])
                nc.sync.dma_start(dinv_d[b, h0:h1], dinv16[h0:h1, 0:N])
                return dinv16

            def emit_norm_bcast_mult(b, oT, dinv16, j):
                dr = norm.tile([128, W], BF16, tag="drep", bufs=2,
                               name=f"dr{j}_{b}")
                for hh in range(2):
                    row = dinv_d[b, 2 * j + hh, :]
                    src = bass.AP(
                        tensor=row.tensor, offset=row.offset,
                        ap=[[0, 64]] + row.ap,
                    )
                    nc.sync.dma_start(dr[hh * 64:(hh + 1) * 64, 0:N], src)
                with nc.allow_low_precision(
                        reason="bf16 normalize multiply at 2x DVE rate"):
                    nc.vector.tensor_tensor(
                        oT[j][:, 0:N], oT[j][:, 0:N], dr[:, 0:N],
                        mybir.AluOpType.mult,
                    )

            def emit_norm(b, oT, dall, pj0=0, pj1=NPAIR, scalar_recip=False):
                dinv16 = emit_norm_recip(b, dall, pj0, pj1,
                                         scalar_recip=scalar_recip)
                for j in range(pj0, pj1):
                    emit_norm_bcast_mult(b, oT, dinv16, j)

            def emit_proj_chunk(b, oT, tt, big=False):
                ts_ = _kr(tt)
                if big:
                    ps = psum_s.tile([128, C], F32, tag="s",
                                     name=f"psp{tt}_{b}")
                else:
                    ps = psum_o.tile([128, C], F32, tag="o",
                                     name=f"psp{tt}_{b}")
                pfirsts = []
                for cc in range(NCC):
                    for (c0, cn) in CG_C:
                        mm = nc.tensor.matmul(
                            ps[0:ts_, c0:c0 + cn],
                            oT[cc][:, tt * 128:tt * 128 + ts_],
                            pw16[cc][:, c0:c0 + cn],
                            start=(cc == 0),
                            stop=(cc == NCC - 1),
                        )
                        if cc == 0 and c0 == 0:
                            pfirsts.append(mm)
                ob = outp.tile([128, C], BF16, tag="ob", bufs=4,
                               name=f"ob{tt}_{b}")
                nc.vector.tensor_tensor(
                    ob[0:ts_, :], ps[0:ts_, :], pb_rep[0:ts_, :],
                    mybir.AluOpType.add,
                )
                nc.gpsimd.dma_start(
                    out_d[b, tt * 128:tt * 128 + ts_, :], ob[0:ts_, :]
                )
                return pfirsts

            # ---- phase A: batch-0 qkv + v, interleaved, PE-dense ----
            xts0 = emit_x0_and_qkvw_interleaved()
            qT0 = [perb.tile([128, W], BF16, tag=f"qT{i}", name=f"qT{i}_0")
                   for i in range(NCC)]
            kT0 = [perb.tile([128, W], BF16, tag=f"kT{i}", name=f"kT{i}_0")
                   for i in range(NCC)]
            vp0 = alloc_vp(0)
            pts00 = [None] * NKC
            pbox00 = [None]
            for i in range(NKC):
                if i < NCC:
                    emit_qkvT_chunk(0, xts0, i, qT0, kT0, evac_vector=False)
                emit_v_convoy(0, xts0, i, vp0, evac_vector=False)
                # start pair-0 attention inside the qkv phase so the serial
                # exp chain begins ~25us earlier; the remaining qkv/v
                # convoys double as its PE filler
                if i == 1:
                    emit_attn_pass1(0, 0, qT0, kT0, range(0, 2), pts00, pbox00)
                elif i == 2:
                    emit_attn_pass1(0, 0, qT0, kT0, range(2, 4), pts00, pbox00)
                elif i == 3:
                    emit_attn_pass1(0, 0, qT0, kT0, range(4, 6), pts00, pbox00)
            emit_vp6_stripes(0, vp0)
            ptails0 = {0: emit_tail_group(0, 0, qT0, kT0)}
            emit_weight_loads_proj()

            # ---- phase B: batch-0 attention; x1/qkv1/v1 as PE filler ----
            oT0 = alloc_oT(0)
            dall0 = norm.tile([12, W], BF16, tag="dall", bufs=1, name="dall_0")
            xts1 = emit_x(1)
            qT1 = [perb.tile([128, W], BF16, tag=f"qT{i}", name=f"qT{i}_1")
                   for i in range(NCC)]
            kT1 = [perb.tile([128, W], BF16, tag=f"kT{i}", name=f"kT{i}_1")
                   for i in range(NCC)]
            vp1 = alloc_vp(1)
            pacers0 = [pbox00[0]]
            pend0 = [pts00]
            for j in range(1, NPAIR):
                pts_j = [None] * NKC
                pbox = [None]
                store = {}
                # fine-grained round-robin: one S kc-tile (2 psum slots) at a
                # time, with independent PE work (O convoys, fillers) between,
                # so the in-order PE queue never parks on a slot-stalled S
                # matmul while ready work exists
                emit_attn_S(0, j, qT0, kT0, range(0, 2), store, pbox)
                emit_attn_expmult(0, j, range(0, 2), store, pts_j)
                emit_attn_pass2(0, j - 1, pend0[j - 1], ptails0,
                                vp0, oT0, dall0, heads=(0,))
                emit_attn_S(0, j, qT0, kT0, range(2, 3), store, pbox)
                emit_attn_expmult(0, j, range(2, 3), store, pts_j)
                emit_attn_pass2(0, j - 1, pend0[j - 1], ptails0,
                                vp0, oT0, dall0, heads=(1,))
                emit_attn_S(0, j, qT0, kT0, range(3, 4), store, pbox)
                emit_attn_expmult(0, j, range(3, 4), store, pts_j)
                # filler: one v1 convoy per pair-phase (psum_o rotation
                # naturally paces it behind the O convoys)
                emit_v_convoy(1, xts1, j - 1, vp1, evac_vector=True)
                emit_attn_S(0, j, qT0, kT0, range(4, 5), store, pbox)
                emit_attn_expmult(0, j, range(4, 5), store, pts_j)
                # filler: one qkv1 chunk per pair-phase (psum_s rotation)
                emit_qkvT_chunk(1, xts1, j - 1, qT1, kT1,
                                evac_vector=True, ocs=(j - 1,))
                emit_attn_S(0, j, qT0, kT0, range(5, 6), store, pbox)
                emit_attn_expmult(0, j, range(5, 6), store, pts_j)
                emit_qkvT_chunk(1, xts1, j - 1, qT1, kT1,
                                evac_vector=True, ocs=(NCC + j - 1,))
                if j == 1:
                    ptails0[1] = emit_tail_group(0, 1, qT0, kT0)
                elif j == 3:
                    ptails0[2] = emit_tail_group(0, 2, qT0, kT0)
                elif j == 4:
                    ptails0[3] = emit_tail_group(0, 3, qT0, kT0)
                pacers0.append(pbox[0])
                pend0.append(pts_j)
            # B->C seam: batch-1 pair-0 attention straddles the last
            # batch-0 O convoys so the exp chain never idles at the seam.
            # (pair-1-pair-0's kc>=1 P tiles reuse slots consumed by
            # pass2(0,5), so only kc=0 may precede it.)
            oT1 = alloc_oT(1)
            dall1 = norm.tile([12, W], BF16, tag="dall", bufs=1, name="dall_1")
            pts10 = [None] * NKC
            pbox10 = [None]
            store10 = {}
            emit_attn_S(1, 0, qT1, kT1, range(0, 1), store10, pbox10)
            emit_attn_expmult(1, 0, range(0, 1), store10, pts10)
            emit_attn_pass2(0, NPAIR - 1, pend0[NPAIR - 1], ptails0,
                            vp0, oT0, dall0)
            emit_attn_S(1, 0, qT1, kT1, range(1, 3), store10, pbox10)
            emit_attn_expmult(1, 0, range(1, 3), store10, pts10)
            emit_norm(0, oT0, dall0)
            # remaining batch-1 prep (chunk 5, v convoys 5-6)
            emit_qkvT_chunk(1, xts1, NCC - 1, qT1, kT1, evac_vector=True)
            emit_attn_S(1, 0, qT1, kT1, range(3, 5), store10, pbox10)
            emit_attn_expmult(1, 0, range(3, 5), store10, pts10)
            for kc in (NKC - 2, NKC - 1):
                emit_v_convoy(1, xts1, kc, vp1, evac_vector=True)
            emit_vp6_stripes(1, vp1)
            emit_attn_S(1, 0, qT1, kT1, range(5, 6), store10, pbox10)
            emit_attn_expmult(1, 0, range(5, 6), store10, pts10)
            ptails1 = {0: emit_tail_group(1, 0, qT1, kT1)}

            # ---- phase C: batch-1 attention; proj0 as PE filler ----
            pacers1 = [pbox10[0]]
            pend1 = [pts10]
            for j in range(1, NPAIR):
                pts_j = [None] * NKC
                pbox = [None]
                store = {}
                emit_attn_S(1, j, qT1, kT1, range(0, 2), store, pbox)
                emit_attn_expmult(1, j, range(0, 2), store, pts_j)
                emit_attn_pass2(1, j - 1, pend1[j - 1], ptails1,
                                vp1, oT1, dall1, heads=(0,))
                emit_attn_S(1, j, qT1, kT1, range(2, 3), store, pbox)
                emit_attn_expmult(1, j, range(2, 3), store, pts_j)
                emit_attn_pass2(1, j - 1, pend1[j - 1], ptails1,
                                vp1, oT1, dall1, heads=(1,))
                emit_attn_S(1, j, qT1, kT1, range(3, 4), store, pbox)
                emit_attn_expmult(1, j, range(3, 4), store, pts_j)
                # fine-grained emission position already places proj0 after
                # this pair's first S tiles; the explicit pacer dep only
                # added a wait that can idle PE
                emit_proj_chunk(0, oT0, j - 1)
                emit_attn_S(1, j, qT1, kT1, range(4, 6), store, pbox)
                emit_attn_expmult(1, j, range(4, 6), store, pts_j)
                if j == 1:
                    ptails1[1] = emit_tail_group(1, 1, qT1, kT1)
                elif j == 3:
                    ptails1[2] = emit_tail_group(1, 2, qT1, kT1)
                elif j == 4:
                    ptails1[3] = emit_tail_group(1, 3, qT1, kT1)
                # incremental norm: normalize completed pairs mid-phase
                # (DVE recip chain, after this pair's mults and the tail
                # group's) so the tail only carries the last pair's chain
                if j == 4:
                    emit_norm(1, oT1, dall1, 0, 3)
                elif j == 5:
                    emit_norm(1, oT1, dall1, 3, 5)
                pacers1.append(pbox[0])
                pend1.append(pts_j)
            emit_attn_pass2(1, NPAIR - 1, pend1[NPAIR - 1], ptails1,
                            vp1, oT1, dall1, heads=(0,))
            emit_proj_chunk(0, oT0, NPAIR - 1, big=True)
            emit_attn_pass2(1, NPAIR - 1, pend1[NPAIR - 1], ptails1,
                            vp1, oT1, dall1, heads=(1,))
            # last pair via the DVE recip chain: the ACT Ln/Exp path costs a
            # table reload + drain (~3.3us) in the critical tail
            emit_norm(1, oT1, dall1, NPAIR - 1, NPAIR)
            for tt in range(NPAIR, NKC):
                emit_proj_chunk(0, oT0, tt, big=True)
            for tt in range(NKC):
                emit_proj_chunk(1, oT1, tt, big=(tt % 2 == 0))

    nc.compile()
    return nc


def _relative_position_index():
    coords = np.stack(np.meshgrid(np.arange(WX), np.arange(WY), indexing="ij"))
    cf = coords.reshape(2, -1)
    rel = cf[:, :, None] - cf[:, None, :]
    rel = rel.transpose(1, 2, 0).astype(np.int64)
    rel[:, :, 0] += WX - 1
    rel[:, :, 1] += WY - 1
    rel[:, :, 0] *= 2 * WY - 1
    return rel.sum(-1)  # [L, L]


def _host_prep(x, qkv_w, proj_w, proj_b, rel_table, g2l, g2g):
    x = np.asarray(x, np.float32)
    qkv_w = np.asarray(qkv_w, np.float32)
    proj_w = np.asarray(proj_w, np.float32)
    proj_b = np.asarray(proj_b, np.float32)
    rel_table = np.asarray(rel_table, np.float32)
    g2l = np.asarray(g2l, np.float32)
    g2g = np.asarray(g2g, np.float32)

    bf16 = ml_dtypes.bfloat16
    xT = np.ascontiguousarray(x.transpose(0, 2, 1)).astype(bf16)   # [B, C, N]
    qkv_wT = np.ascontiguousarray(qkv_w.T).copy()                  # [C, 3C]
    qkv_wT[:, :C] *= SCALE                                         # fold q scale
    qkv_wT = qkv_wT.astype(bf16)
    proj_wT = np.ascontiguousarray(proj_w.T).astype(bf16)          # [C, C]
    pb = proj_b.reshape(1, C).astype(bf16)

    # expB[h, k, q] = exp(bias[h, q, k]); exp applied at table granularity,
    # then expanded by the constant-index relative-position gather.
    ridx = _relative_position_index()
    et = np.exp(rel_table)                                         # [3025, H]
    eg2l = np.exp(g2l)                                             # [2, H, 1]
    eg2g = np.exp(g2g)                                             # [H, 1, 1]
    expB = np.empty((H, N, N), np.float32)
    expB[:, 1:, 1:] = et[ridx].transpose(2, 1, 0)                  # [H, k, q]
    expB[:, 0, 0] = eg2g[:, 0, 0]
    expB[:, 1:, 0] = eg2l[0][:, 0][None, :].T                      # global query
    expB[:, 0, 1:] = eg2l[1][:, 0][:, None]                        # global key
    expB16 = expB.astype(bf16)

    in_maps = []
    for i in range(N_CORES):
        in_maps.append({
            "xT": xT[i * B_LOC:(i + 1) * B_LOC],
            "qkv_wT": qkv_wT,
            "proj_wT": proj_wT,
            "proj_b": pb,
            "expB": expB16,
        })
    return in_maps


_NC = None


def get_nc():
    global _NC
    if _NC is None:
        _NC = build_nc()
    return _NC


def kernel(x, qkv_w, proj_w, proj_b, rel_table, g2l, g2g):
    in_maps = _host_prep(x, qkv_w, proj_w, proj_b, rel_table, g2l, g2g)
    nc = get_nc()
    res = run_bass_kernel_spmd(nc, in_maps, core_ids=list(range(N_CORES)))
    out = np.concatenate([res.results[i]["out"] for i in range(N_CORES)], axis=0)
    return out.astype(np.float32)



# revision 83
# speedup vs baseline: 1.0134x; 1.0134x over previous
"""Trainium2 Bass kernel for windowed/global sparse attention (Swin-style
relative-position bias + 1 global token), data-parallel over batch on 8 cores.

Shapes: B=16, N=785 (1 global + 28x28 local), C=768, H=12 heads, d=64.

Per-core device program (2 batches/core, software-pipelined):
  - qT/kT computed transposed ([d, tokens]) so S^T = K @ Q^T needs no
    transposes anywhere; v computed natural ([tokens, d]) with a ones column
    appended per head so the P @ V matmul also yields softmax denominators.
  - softmax: exp(S + bias) = exp(S) * expB with expB = exp(bias) gathered on
    host at constant indices and shipped bf16; the two heads of a pair write
    one fused [128, 2W] SBUF exp tile so the expB multiply is a single
    2x-rate DVE op.
  - PSUM: 3 rotating 2-bank slots for S tiles (and qkv convoys) + one
    dedicated slot for O/v/proj convoys, decoupling the S stream from the O
    convoys.
  - normalization: denominators from all 12 heads staged to DRAM, one batched
    DVE reciprocal, DMA-broadcast back to [128, N], multiplied into O^T;
    proj consumes O^T directly as lhsT.
  - schedule: [qkv0 || v0] dense, then attention-0 with x1/qkv1/v1 as PE
    gap-filler, attention-1 with proj0 as filler, then norm1+proj1 - keeps
    the PE activity monitor from re-throttling the clock during the
    exp-paced attention stretches.
"""

import numpy as np
import ml_dtypes

import concourse.bass as bass
import concourse.bacc as bacc
import concourse.tile as tile
from concourse.tile import add_dep_helper
from concourse import mybir
from concourse.bass_utils import run_bass_kernel_spmd

F32 = mybir.dt.float32
BF16 = mybir.dt.bfloat16

WX = WY = 28
NGLO = 1
H = 12
L = WX * WY            # 784
N = NGLO + L           # 785
C = 768
HD = C // H            # 64
SCALE = HD ** -0.5
B = 16
N_CORES = 8
B_LOC = B // N_CORES   # 2
NCC = C // 128         # 6 contraction chunks
NKC = (N + 127) // 128  # 7 key/token chunks (last = 17 rows)
NPAIR = H // 2         # 6 head pairs
W = 786                # padded free width for N-sized tiles (even, 4B-aligned)
W2 = 2 * W

CG_N = [(0, 512), (512, 274)]
CG_C = [(0, 512), (512, 256)]


def _kr(kc):
    return min(128, N - kc * 128)


def build_nc():
    nc = bacc.Bacc(None, target_bir_lowering=False)

    xT_d = nc.dram_tensor("xT", [B_LOC, C, N], BF16, kind="ExternalInput")
    qkvwT_d = nc.dram_tensor("qkv_wT", [C, 3 * C], BF16, kind="ExternalInput")
    pwT_d = nc.dram_tensor("proj_wT", [C, C], BF16, kind="ExternalInput")
    pb_d = nc.dram_tensor("proj_b", [1, C], BF16, kind="ExternalInput")
    expB_d = nc.dram_tensor("expB", [H, N, N], BF16, kind="ExternalInput")
    out_d = nc.dram_tensor("out", [B_LOC, N, C], BF16, kind="ExternalOutput")
    dinv_d = nc.dram_tensor("dinv_scratch", [B_LOC, H, N], BF16)

    with tile.TileContext(nc) as tc:
        with (
            tc.tile_pool(name="consts", bufs=1) as consts,
            tc.tile_pool(name="perb", bufs=2) as perb,
            tc.tile_pool(name="expbp", bufs=3) as expbp,
            tc.tile_pool(name="flow", bufs=4) as flow,
            tc.tile_pool(name="ptp", bufs=8) as ptp,
            tc.tile_pool(name="norm", bufs=1) as norm,
            tc.tile_pool(name="outp", bufs=2) as outp,
            tc.tile_pool(name="psum_s", bufs=3, space=bass.MemorySpace.PSUM) as psum_s,
            tc.tile_pool(name="psum_o", bufs=1, space=bass.MemorySpace.PSUM) as psum_o,
        ):
            # ---- weights (resident, bf16); proj weights loaded last ----
            qkvw = []
            for cc in range(NCC):
                t = consts.tile([128, 3 * C], BF16, tag=f"qkvw{cc}", name=f"qkvw{cc}")
                qkvw.append(t)
            pw16 = []
            for cc in range(NCC):
                t = consts.tile([128, C], BF16, tag=f"pw{cc}", name=f"pw{cc}")
                pw16.append(t)
            pb_rep = consts.tile([128, C], BF16, tag="pbrep")

            def emit_weight_loads_proj():
                # proj weights aren't needed until phase C; issue them on the
                # Pool queue to keep sync free for the expB stream.
                for cc in range(NCC):
                    nc.gpsimd.dma_start(
                        pw16[cc][:], pwT_d[cc * 128:(cc + 1) * 128, :]
                    )
                nc.gpsimd.dma_start(pb_rep[:], pb_d[:].to_broadcast([128, C]))

            def emit_x(b, eng=None):
                # pad column [N:W] left as garbage: it only ever feeds the
                # q=785 / token=785 output columns, which are never read.
                eng = eng or nc.sync
                xts = []
                for cc in range(NCC):
                    t = perb.tile([128, W], BF16, tag=f"xt{cc}", name=f"xt{cc}_{b}")
                    eng.dma_start(
                        t[:, 0:N], xT_d[b, cc * 128:(cc + 1) * 128, :]
                    )
                    xts.append(t)
                return xts

            def emit_x0_and_qkvw_interleaved():
                # startup critical path: the first qkv matmuls need the q/k
                # weight columns and x chunks in cc order; v columns aren't
                # touched until the first v convoy, so defer them. Interleave
                # so chunk 0 of everything lands first.
                xts = []
                for cc in range(NCC):
                    nc.sync.dma_start(
                        qkvw[cc][:], qkvwT_d[cc * 128:(cc + 1) * 128, :]
                    )
                    t = perb.tile([128, W], BF16, tag=f"xt{cc}", name=f"xt{cc}_0")
                    nc.scalar.dma_start(
                        t[:, 0:N], xT_d[0, cc * 128:(cc + 1) * 128, :]
                    )
                    xts.append(t)
                return xts

            def emit_qkvT_chunk(b, xts, j, qT, kT, evac_vector=False,
                                defer=False, ocs=None):
                """produce qT[j] and kT[j] for batch b."""
                firsts = []
                evacs = []
                for oc in (ocs if ocs is not None else (j, NCC + j)):
                    ps = psum_s.tile([128, W], F32, tag="s", name=f"psqk{oc}_{b}")
                    for cc in range(NCC):
                        for (c0, cn) in CG_N:
                            mm = nc.tensor.matmul(
                                ps[:, c0:c0 + cn],
                                qkvw[cc][:, oc * 128:(oc + 1) * 128],
                                xts[cc][:, c0:c0 + cn],
                                start=(cc == 0),
                                stop=(cc == NCC - 1),
                            )
                            if cc == 0 and c0 == 0:
                                firsts.append(mm)
                    dst = qT[oc] if oc < NCC else kT[oc - NCC]

                    def ev(dst=dst, ps=ps):
                        if evac_vector:
                            nc.vector.tensor_copy(dst[:, 0:W], ps[:, 0:W])
                        else:
                            nc.scalar.copy(dst[:, 0:W], ps[:, 0:W])
                    if defer:
                        evacs.append(ev)
                    else:
                        ev()
                if defer:
                    return firsts, evacs
                return firsts

            def emit_v_convoy(b, xts, kc, vp, evac_vector, defer=False):
                """one key-chunk's V matmul convoy + evac into vp[kc]."""
                kr = _kr(kc)
                ps = psum_o.tile([128, C], F32, tag="o", name=f"psv{kc}_{b}")
                first = None
                for cc in range(NCC):
                    for (c0, cn) in CG_C:
                        mm = nc.tensor.matmul(
                            ps[0:kr, c0:c0 + cn],
                            xts[cc][:, kc * 128:kc * 128 + kr],
                            qkvw[cc][:, 2 * C + c0:2 * C + c0 + cn],
                            start=(cc == 0),
                            stop=(cc == NCC - 1),
                        )
                        if first is None:
                            first = mm
                v3 = vp[kc][:].rearrange("p (h e) -> p h e", e=HD + 1)

                def ev():
                    if evac_vector:
                        nc.vector.tensor_copy(
                            v3[0:kr, :, 0:HD],
                            ps[0:kr, :].rearrange("p (h d) -> p h d", d=HD),
                        )
                    else:
                        nc.scalar.copy(
                            v3[0:kr, :, 0:HD],
                            ps[0:kr, :].rearrange("p (h d) -> p h d", d=HD),
                        )
                    nc.gpsimd.memset(v3[0:kr, :, HD:HD + 1], 1.0)
                if defer:
                    return first, ev
                ev()
                return first

            def alloc_vp(b):
                return [perb.tile([128, H * (HD + 1)], BF16, tag=f"vp{i}",
                                  name=f"vp{i}_{b}") for i in range(NKC)]

            def alloc_oT(b):
                return [perb.tile([128, W], BF16, tag=f"oT{i}", name=f"oT{i}_{b}")
                        for i in range(NCC)]

            def emit_attn_S(b, j, qT, kT, kcs, store, pacer_box):
                """S matmuls + ebt DMA only (PE + sync streams) for head pair
                (2j, 2j+1); exp/mult emitted separately so PE-ready S work
                can be emitted ahead of O convoys without perturbing the
                ACT/DVE instruction order."""
                for kc in kcs:
                    kr = _kr(kc)
                    ps_pair = [
                        psum_s.tile([128, W], F32, tag="s",
                                    name=f"pss{2 * j + hh}_{kc}_{b}")
                        for hh in range(2)
                    ]
                    for (c0, cn) in CG_N:
                        for hh in range(2):
                            po = hh * 64
                            mm = nc.tensor.matmul(
                                ps_pair[hh][0:kr, c0:c0 + cn],
                                kT[j][po:po + 64, kc * 128:kc * 128 + kr],
                                qT[j][po:po + 64, c0:c0 + cn],
                                start=True,
                                stop=True,
                            )
                            if kc == 2 and pacer_box[0] is None:
                                pacer_box[0] = mm
                    ebt = expbp.tile([128, W2], BF16, tag="expb",
                                     name=f"ebt{j}_{kc}_{b}")
                    # one fused DMA for both heads of the pair: dst viewed as
                    # [kr, 2, W], src as [kr, 2, N] — halves HWDGE issue work
                    ebt3 = ebt[0:kr, :].rearrange("k (h w) -> k h w", w=W)
                    src3 = expB_d[
                        2 * j:2 * j + 2, kc * 128:kc * 128 + kr, :
                    ].rearrange("h k n -> k h n")
                    nc.sync.dma_start(ebt3[:, :, 0:N], src3)
                    store[kc] = (ps_pair, ebt)

            def emit_attn_expmult(b, j, kcs, store, pts):
                """exp (ACT) + fused expB multiply (DVE) for staged S tiles."""
                for kc in kcs:
                    kr = _kr(kc)
                    ps_pair, ebt = store[kc]
                    es = flow.tile([128, W2], BF16, tag="expS",
                                   name=f"es{j}_{kc}_{b}")
                    for hh in range(2):
                        nc.scalar.activation(
                            es[0:kr, hh * W:(hh + 1) * W],
                            ps_pair[hh][0:kr, 0:W],
                            mybir.ActivationFunctionType.Exp,
                        )
                    pt = ptp.tile([128, W2], BF16, tag="pT",
                                  name=f"pt{j}_{kc}_{b}")
                    nc.vector.tensor_tensor(
                        pt[0:kr, 0:W2],
                        es[0:kr, 0:W2],
                        ebt[0:kr, 0:W2],
                        mybir.AluOpType.mult,
                    )
                    pts[kc] = pt

            def emit_attn_pass1(b, j, qT, kT, kcs, pts, pacer_box):
                store = {}
                emit_attn_S(b, j, qT, kT, kcs, store, pacer_box)
                emit_attn_expmult(b, j, kcs, store, pts)

            def emit_tail_group(b, g, qT, kT):
                """kc=6 tail (17 k-rows) for three heads 3g..3g+2, packed at
                partition stripes {0,32,64} of ONE psum tile: one exp and
                one expB-multiply instead of three of each. Gap stripes hold
                garbage that is never read downstream."""
                ps_t = psum_s.tile([128, W], F32, tag="s",
                                   name=f"pstail{g}_{b}")
                for idx in range(3):
                    h = 3 * g + idx
                    j, po, p0 = h // 2, 64 * (h % 2), 32 * idx
                    for (c0, cn) in CG_N:
                        nc.tensor.matmul(
                            ps_t[p0:p0 + 17, c0:c0 + cn],
                            kT[j][po:po + 64, 6 * 128:N],
                            qT[j][po:po + 64, c0:c0 + cn],
                            start=True,
                            stop=True,
                        )
                ebt_t = expbp.tile([128, W], BF16, tag="expbt", bufs=1,
                                   name=f"ebtail{g}_{b}")
                for idx in range(3):
                    h = 3 * g + idx
                    p0 = 32 * idx
                    nc.sync.dma_start(
                        ebt_t[p0:p0 + 17, 0:N], expB_d[h, 6 * 128:N, :]
                    )
                es_t = flow.tile([128, W], BF16, tag="expSt", bufs=1,
                                 name=f"estail{g}_{b}")
                nc.scalar.activation(
                    es_t[0:81, 0:W], ps_t[0:81, 0:W],
                    mybir.ActivationFunctionType.Exp,
                )
                pt_t = ptp.tile([128, W], BF16, tag="pTt", bufs=2,
                                name=f"pttail{g}_{b}")
                nc.vector.tensor_tensor(
                    pt_t[0:81, 0:W], es_t[0:81, 0:W], ebt_t[0:81, 0:W],
                    mybir.AluOpType.mult,
                )
                return pt_t

            def emit_vp6_stripes(b, vp):
                # replicate the 17 tail V rows (and their ones column) to the
                # 32-aligned stripes the packed tail P tiles live at, so the
                # kc=6 O matmuls see matching operand partition bases
                for idx in range(1, 3):
                    nc.sync.dma_start(
                        vp[6][32 * idx:32 * idx + 17, :], vp[6][0:17, :]
                    )

            def emit_attn_pass2(b, j, pts, ptails, vp, oT, dall, heads=(0, 1)):
                """dense O-accumulation convoy for head pair (2j, 2j+1).
                kc<6 P tiles are per-pair [128, 2W]; the kc=6 tail P comes
                from the packed 3-head tile ptails[h//3] at stripe 32*(h%3).
                `heads` selects which of the pair's heads to emit, so the
                two convoys can be interleaved with other PE work."""
                for hh in heads:
                    h = 2 * j + hh
                    p0 = 32 * (h % 3)
                    ptail = ptails[h // 3]
                    ps_o = psum_o.tile([128, W], F32, tag="o",
                                       name=f"pso{h}_{b}")
                    for kc in range(NKC):
                        kr = _kr(kc)
                        for (c0, cn) in CG_N:
                            if kc < NKC - 1:
                                lhsT = vp[kc][0:kr,
                                              h * (HD + 1):(h + 1) * (HD + 1)]
                                rhs = pts[kc][0:kr,
                                              hh * W + c0:hh * W + c0 + cn]
                            else:
                                lhsT = vp[kc][p0:p0 + kr,
                                              h * (HD + 1):(h + 1) * (HD + 1)]
                                rhs = ptail[p0:p0 + kr, c0:c0 + cn]
                            nc.tensor.matmul(
                                ps_o[0:HD + 1, c0:c0 + cn],
                                lhsT,
                                rhs,
                                start=(kc == 0),
                                stop=(kc == NKC - 1),
                            )
                    if hh == 0:
                        # denominator row rides along in the oT evac (row 64
                        # is head B's territory, but the dall DMA reads it
                        # before head B's evac overwrites — WAR-ordered)
                        nc.vector.tensor_copy(
                            oT[j][0:65, 0:N], ps_o[0:65, 0:N]
                        )
                        nc.sync.dma_start(
                            dall[h:h + 1, 0:N], oT[j][64:65, 0:N]
                        )
                    else:
                        nc.vector.tensor_copy(
                            oT[j][64:128, 0:N], ps_o[0:64, 0:N]
                        )
                        dn = norm.tile([65, W], BF16, tag="dn", bufs=1,
                                       name=f"dn{h}_{b}")
                        nc.vector.tensor_copy(dn[64:65, 0:N],
                                              ps_o[64:65, 0:N])
                        nc.sync.dma_start(dall[h:h + 1, 0:N],
                                          dn[64:65, 0:N])

            def emit_norm_recip(b, dall, pj0, pj1, scalar_recip=False):
                """1/den for pairs [pj0, pj1) into a bf16 dinv tile.
                DVE path: cast->recip_approx->cast (no ACT involvement).
                ACT path (tail only, ACT drained): 1/x = exp(-ln x)."""
                # engines need 32-aligned partition bases; recomputing rows
                # 0..h0 is free (cost is free-size-bound), so start at 0
                h0, h1 = 0, 2 * pj1
                dinv16 = norm.tile([12, W], BF16, tag="dinv16",
                                   name=f"dinv16_{pj0}_{b}", bufs=1)
                if scalar_recip:
                    lnt = norm.tile([12, W], F32, tag="dall32", bufs=1,
                                    name=f"lnt_{pj0}_{b}")
                    nc.scalar.activation(
                        lnt[h0:h1, 0:N], dall[h0:h1, 0:N],
                        mybir.ActivationFunctionType.Ln,
                    )
                    nc.scalar.activation(
                        dinv16[h0:h1, 0:N], lnt[h0:h1, 0:N],
                        mybir.ActivationFunctionType.Exp,
                        scale=-1.0,
                    )
                else:
                    dall32 = norm.tile([12, W], F32, tag="dall32",
                                       name=f"dall32_{pj0}_{b}", bufs=1)
                    dinv32 = norm.tile([12, W], F32, tag="dinv32",
                                       name=f"dinv32_{pj0}_{b}", bufs=1)
                    nc.vector.tensor_copy(dall32[h0:h1, 0:N],
                                          dall[h0:h1, 0:N])
                    nc.vector.reciprocal_approx_fast(
                        dinv32[h0:h1, 0:N], dall32[h0:h1, 0:N])
                    with nc.allow_low_precision(
                            reason="1/den in bf16: uniform per-row scale, "
                                   "~0.2% rms, well inside the 2e-2 gate"):
                        nc.vector.tensor_copy(dinv16[h0:h1, 0:N],
                                              dinv32[h0:h1, 0:N])
                nc.sync.dma_start(dinv_d[b, h0:h1], dinv16[h0:h1, 0:N])
                return dinv16

            def emit_norm_bcast_mult(b, oT, dinv16, j):
                dr = norm.tile([128, W], BF16, tag="drep", bufs=2,
                               name=f"dr{j}_{b}")
                for hh in range(2):
                    row = dinv_d[b, 2 * j + hh, :]
                    src = bass.AP(
                        tensor=row.tensor, offset=row.offset,
                        ap=[[0, 64]] + row.ap,
                    )
                    nc.sync.dma_start(dr[hh * 64:(hh + 1) * 64, 0:N], src)
                with nc.allow_low_precision(
                        reason="bf16 normalize multiply at 2x DVE rate"):
                    nc.vector.tensor_tensor(
                        oT[j][:, 0:N], oT[j][:, 0:N], dr[:, 0:N],
                        mybir.AluOpType.mult,
                    )

            def emit_norm(b, oT, dall, pj0=0, pj1=NPAIR, scalar_recip=False):
                dinv16 = emit_norm_recip(b, dall, pj0, pj1,
                                         scalar_recip=scalar_recip)
                for j in range(pj0, pj1):
                    emit_norm_bcast_mult(b, oT, dinv16, j)

            def emit_proj_chunk(b, oT, tt, big=False):
                ts_ = _kr(tt)
                if big:
                    ps = psum_s.tile([128, C], F32, tag="s",
                                     name=f"psp{tt}_{b}")
                else:
                    ps = psum_o.tile([128, C], F32, tag="o",
                                     name=f"psp{tt}_{b}")
                pfirsts = []
                for cc in range(NCC):
                    for (c0, cn) in CG_C:
                        mm = nc.tensor.matmul(
                            ps[0:ts_, c0:c0 + cn],
                            oT[cc][:, tt * 128:tt * 128 + ts_],
                            pw16[cc][:, c0:c0 + cn],
                            start=(cc == 0),
                            stop=(cc == NCC - 1),
                        )
                        if cc == 0 and c0 == 0:
                            pfirsts.append(mm)
                ob = outp.tile([128, C], BF16, tag="ob", bufs=4,
                               name=f"ob{tt}_{b}")
                nc.vector.tensor_tensor(
                    ob[0:ts_, :], ps[0:ts_, :], pb_rep[0:ts_, :],
                    mybir.AluOpType.add,
                )
                nc.gpsimd.dma_start(
                    out_d[b, tt * 128:tt * 128 + ts_, :], ob[0:ts_, :]
                )
                return pfirsts

            # ---- phase A: batch-0 qkv + v, interleaved, PE-dense ----
            xts0 = emit_x0_and_qkvw_interleaved()
            qT0 = [perb.tile([128, W], BF16, tag=f"qT{i}", name=f"qT{i}_0")
                   for i in range(NCC)]
            kT0 = [perb.tile([128, W], BF16, tag=f"kT{i}", name=f"kT{i}_0")
                   for i in range(NCC)]
            vp0 = alloc_vp(0)
            pts00 = [None] * NKC
            pbox00 = [None]
            for i in range(NKC):
                if i < NCC:
                    emit_qkvT_chunk(0, xts0, i, qT0, kT0, evac_vector=False)
                emit_v_convoy(0, xts0, i, vp0, evac_vector=False)
                # start pair-0 attention inside the qkv phase so the serial
                # exp chain begins ~25us earlier; the remaining qkv/v
                # convoys double as its PE filler
                if i == 1:
                    emit_attn_pass1(0, 0, qT0, kT0, range(0, 2), pts00, pbox00)
                elif i == 2:
                    emit_attn_pass1(0, 0, qT0, kT0, range(2, 4), pts00, pbox00)
                elif i == 3:
                    emit_attn_pass1(0, 0, qT0, kT0, range(4, 6), pts00, pbox00)
            emit_vp6_stripes(0, vp0)
            ptails0 = {0: emit_tail_group(0, 0, qT0, kT0)}
            # prefetch batch-1 activations on the idle Pool queue while
            # sync is still quiet (phase B's sync queue carries the expB
            # stream)
            xts1 = emit_x(1, eng=nc.gpsimd)
            emit_weight_loads_proj()

            # ---- phase B: batch-0 attention; x1/qkv1/v1 as PE filler ----
            oT0 = alloc_oT(0)
            dall0 = norm.tile([12, W], BF16, tag="dall", bufs=1, name="dall_0")
            qT1 = [perb.tile([128, W], BF16, tag=f"qT{i}", name=f"qT{i}_1")
                   for i in range(NCC)]
            kT1 = [perb.tile([128, W], BF16, tag=f"kT{i}", name=f"kT{i}_1")
                   for i in range(NCC)]
            vp1 = alloc_vp(1)
            pacers0 = [pbox00[0]]
            pend0 = [pts00]
            for j in range(1, NPAIR):
                pts_j = [None] * NKC
                pbox = [None]
                store = {}
                # fine-grained round-robin: one S kc-tile (2 psum slots) at a
                # time, with independent PE work (O convoys, fillers) between,
                # so the in-order PE queue never parks on a slot-stalled S
                # matmul while ready work exists
                emit_attn_S(0, j, qT0, kT0, range(0, 2), store, pbox)
                emit_attn_expmult(0, j, range(0, 2), store, pts_j)
                emit_attn_pass2(0, j - 1, pend0[j - 1], ptails0,
                                vp0, oT0, dall0, heads=(0,))
                emit_attn_S(0, j, qT0, kT0, range(2, 3), store, pbox)
                emit_attn_expmult(0, j, range(2, 3), store, pts_j)
                emit_attn_pass2(0, j - 1, pend0[j - 1], ptails0,
                                vp0, oT0, dall0, heads=(1,))
                emit_attn_S(0, j, qT0, kT0, range(3, 4), store, pbox)
                emit_attn_expmult(0, j, range(3, 4), store, pts_j)
                # filler: one v1 convoy per pair-phase (psum_o rotation
                # naturally paces it behind the O convoys)
                emit_v_convoy(1, xts1, j - 1, vp1, evac_vector=True)
                emit_attn_S(0, j, qT0, kT0, range(4, 5), store, pbox)
                emit_attn_expmult(0, j, range(4, 5), store, pts_j)
                # filler: one qkv1 chunk per pair-phase (psum_s rotation)
                emit_qkvT_chunk(1, xts1, j - 1, qT1, kT1,
                                evac_vector=True, ocs=(j - 1,))
                emit_attn_S(0, j, qT0, kT0, range(5, 6), store, pbox)
                emit_attn_expmult(0, j, range(5, 6), store, pts_j)
                emit_qkvT_chunk(1, xts1, j - 1, qT1, kT1,
                                evac_vector=True, ocs=(NCC + j - 1,))
                if j == 1:
                    ptails0[1] = emit_tail_group(0, 1, qT0, kT0)
                elif j == 3:
                    ptails0[2] = emit_tail_group(0, 2, qT0, kT0)
                elif j == 4:
                    ptails0[3] = emit_tail_group(0, 3, qT0, kT0)
                pacers0.append(pbox[0])
                pend0.append(pts_j)
            # B->C seam: batch-1 pair-0 attention straddles the last
            # batch-0 O convoys so the exp chain never idles at the seam.
            # (pair-1-pair-0's kc>=1 P tiles reuse slots consumed by
            # pass2(0,5), so only kc=0 may precede it.)
            oT1 = alloc_oT(1)
            dall1 = norm.tile([12, W], BF16, tag="dall", bufs=1, name="dall_1")
            pts10 = [None] * NKC
            pbox10 = [None]
            store10 = {}
            emit_attn_S(1, 0, qT1, kT1, range(0, 1), store10, pbox10)
            emit_attn_expmult(1, 0, range(0, 1), store10, pts10)
            emit_attn_pass2(0, NPAIR - 1, pend0[NPAIR - 1], ptails0,
                            vp0, oT0, dall0)
            emit_attn_S(1, 0, qT1, kT1, range(1, 3), store10, pbox10)
            emit_attn_expmult(1, 0, range(1, 3), store10, pts10)
            emit_norm(0, oT0, dall0)
            # remaining batch-1 prep (chunk 5, v convoys 5-6)
            emit_qkvT_chunk(1, xts1, NCC - 1, qT1, kT1, evac_vector=True)
            emit_attn_S(1, 0, qT1, kT1, range(3, 5), store10, pbox10)
            emit_attn_expmult(1, 0, range(3, 5), store10, pts10)
            for kc in (NKC - 2, NKC - 1):
                emit_v_convoy(1, xts1, kc, vp1, evac_vector=True)
            emit_vp6_stripes(1, vp1)
            emit_attn_S(1, 0, qT1, kT1, range(5, 6), store10, pbox10)
            emit_attn_expmult(1, 0, range(5, 6), store10, pts10)
            ptails1 = {0: emit_tail_group(1, 0, qT1, kT1)}

            # ---- phase C: batch-1 attention; proj0 as PE filler ----
            pacers1 = [pbox10[0]]
            pend1 = [pts10]
            for j in range(1, NPAIR):
                pts_j = [None] * NKC
                pbox = [None]
                store = {}
                emit_attn_S(1, j, qT1, kT1, range(0, 2), store, pbox)
                emit_attn_expmult(1, j, range(0, 2), store, pts_j)
                emit_attn_pass2(1, j - 1, pend1[j - 1], ptails1,
                                vp1, oT1, dall1, heads=(0,))
                emit_attn_S(1, j, qT1, kT1, range(2, 3), store, pbox)
                emit_attn_expmult(1, j, range(2, 3), store, pts_j)
                emit_attn_pass2(1, j - 1, pend1[j - 1], ptails1,
                                vp1, oT1, dall1, heads=(1,))
                emit_attn_S(1, j, qT1, kT1, range(3, 4), store, pbox)
                emit_attn_expmult(1, j, range(3, 4), store, pts_j)
                # fine-grained emission position already places proj0 after
                # this pair's first S tiles; the explicit pacer dep only
                # added a wait that can idle PE
                emit_proj_chunk(0, oT0, j - 1)
                emit_attn_S(1, j, qT1, kT1, range(4, 6), store, pbox)
                emit_attn_expmult(1, j, range(4, 6), store, pts_j)
                if j == 1:
                    ptails1[1] = emit_tail_group(1, 1, qT1, kT1)
                elif j == 3:
                    ptails1[2] = emit_tail_group(1, 2, qT1, kT1)
                elif j == 4:
                    ptails1[3] = emit_tail_group(1, 3, qT1, kT1)
                # incremental norm: normalize completed pairs mid-phase
                # (DVE recip chain, after this pair's mults and the tail
                # group's) so the tail only carries the last pair's chain
                if j == 4:
                    emit_norm(1, oT1, dall1, 0, 3)
                elif j == 5:
                    emit_norm(1, oT1, dall1, 3, 5)
                pacers1.append(pbox[0])
                pend1.append(pts_j)
            emit_attn_pass2(1, NPAIR - 1, pend1[NPAIR - 1], ptails1,
                            vp1, oT1, dall1, heads=(0,))
            emit_proj_chunk(0, oT0, NPAIR - 1, big=True)
            emit_attn_pass2(1, NPAIR - 1, pend1[NPAIR - 1], ptails1,
                            vp1, oT1, dall1, heads=(1,))
            # last pair via the DVE recip chain: the ACT Ln/Exp path costs a
            # table reload + drain (~3.3us) in the critical tail
            emit_norm(1, oT1, dall1, NPAIR - 1, NPAIR)
            for tt in range(NPAIR, NKC):
                emit_proj_chunk(0, oT0, tt, big=True)
            for tt in range(NKC):
                emit_proj_chunk(1, oT1, tt, big=(tt % 2 == 0))

    nc.compile()
    return nc


def _relative_position_index():
    coords = np.stack(np.meshgrid(np.arange(WX), np.arange(WY), indexing="ij"))
    cf = coords.reshape(2, -1)
    rel = cf[:, :, None] - cf[:, None, :]
    rel = rel.transpose(1, 2, 0).astype(np.int64)
    rel[:, :, 0] += WX - 1
    rel[:, :, 1] += WY - 1
    rel[:, :, 0] *= 2 * WY - 1
    return rel.sum(-1)  # [L, L]


def _host_prep(x, qkv_w, proj_w, proj_b, rel_table, g2l, g2g):
    x = np.asarray(x, np.float32)
    qkv_w = np.asarray(qkv_w, np.float32)
    proj_w = np.asarray(proj_w, np.float32)
    proj_b = np.asarray(proj_b, np.float32)
    rel_table = np.asarray(rel_table, np.float32)
    g2l = np.asarray(g2l, np.float32)
    g2g = np.asarray(g2g, np.float32)

    bf16 = ml_dtypes.bfloat16
    xT = np.ascontiguousarray(x.transpose(0, 2, 1)).astype(bf16)   # [B, C, N]
    qkv_wT = np.ascontiguousarray(qkv_w.T).copy()                  # [C, 3C]
    qkv_wT[:, :C] *= SCALE                                         # fold q scale
    qkv_wT = qkv_wT.astype(bf16)
    proj_wT = np.ascontiguousarray(proj_w.T).astype(bf16)          # [C, C]
    pb = proj_b.reshape(1, C).astype(bf16)

    # expB[h, k, q] = exp(bias[h, q, k]); exp applied at table granularity,
    # then expanded by the constant-index relative-position gather.
    ridx = _relative_position_index()
    et = np.exp(rel_table)                                         # [3025, H]
    eg2l = np.exp(g2l)                                             # [2, H, 1]
    eg2g = np.exp(g2g)                                             # [H, 1, 1]
    expB = np.empty((H, N, N), np.float32)
    expB[:, 1:, 1:] = et[ridx].transpose(2, 1, 0)                  # [H, k, q]
    expB[:, 0, 0] = eg2g[:, 0, 0]
    expB[:, 1:, 0] = eg2l[0][:, 0][None, :].T                      # global query
    expB[:, 0, 1:] = eg2l[1][:, 0][:, None]                        # global key
    expB16 = expB.astype(bf16)

    in_maps = []
    for i in range(N_CORES):
        in_maps.append({
            "xT": xT[i * B_LOC:(i + 1) * B_LOC],
            "qkv_wT": qkv_wT,
            "proj_wT": proj_wT,
            "proj_b": pb,
            "expB": expB16,
        })
    return in_maps


_NC = None


def get_nc():
    global _NC
    if _NC is None:
        _NC = build_nc()
    return _NC


def kernel(x, qkv_w, proj_w, proj_b, rel_table, g2l, g2g):
    in_maps = _host_prep(x, qkv_w, proj_w, proj_b, rel_table, g2l, g2g)
    nc = get_nc()
    res = run_bass_kernel_spmd(nc, in_maps, core_ids=list(range(N_CORES)))
    out = np.concatenate([res.results[i]["out"] for i in range(N_CORES)], axis=0)
    return out.astype(np.float32)

